# revision 5
# baseline (speedup 1.0000x reference)
"""Causal self-attention on 8 NeuronCores (Bass/Tile, fp32r matmuls).

Sharding: tensor-parallel over heads x data-parallel over batch.
  core c -> batch b = c//4, heads 4g..4g+3 where g = c%4.
Each core computes q,k,v for its 4 heads (over its batch's 2048 tokens),
causal softmax attention in transposed-score layout [k, q] (denominator via
an extra ones-column on v), and the partial output projection over its 256
head-channels. Host sums the 4 partials per batch and adds b_proj.

Matmuls run as float32r (full PE rate at N>=256, ~1e-4 relative rounding);
the attention probabilities p and values v are bf16 (DVE 2x/4x modes; the
softmax numerator and denominator use the same rounded p, so the error
largely cancels). The 1/sqrt(d) scale is folded into W_k/b_k on the host.

The per-512-token stripes are emitted interleaved (qkv stripe ti, then
attention stripe qi=ti, then its output projection) so the Tile scheduler
overlaps PE-heavy projection work with ACT-heavy softmax work.
"""

import os
import sys

for _p in ("/opt/trn_rl_repo", "/opt/pypackages"):
    if os.path.isdir(_p) and _p not in sys.path:
        sys.path.append(_p)

import numpy as np

import concourse.bass as bass
import concourse.tile as tile
import concourse.mybir as mybir
from concourse import bacc
from concourse.bass_utils import run_bass_kernel_spmd

B, T, C = 2, 2048, 1024
H = 16            # total heads
D = 64            # head dim
HPC = 4           # heads per core
CH = HPC * D      # 256 channels per core
N_CORES = 8

f32 = mybir.dt.float32
f32r = mybir.dt.float32r
bf16 = mybir.dt.bfloat16
ts = bass.ts

_COMPILED = None


def _build():
    nc = bacc.Bacc("TRN2", target_bir_lowering=False, debug=False,
                   num_devices=N_CORES)

    xT = nc.dram_tensor("xT", [C, T], f32, kind="ExternalInput").ap()
    wt = nc.dram_tensor("wt", [C, 3 * CH], f32, kind="ExternalInput").ap()
    wpt = nc.dram_tensor("wpt", [CH, C], f32, kind="ExternalInput").ap()
    bqk = nc.dram_tensor("bqk", [128, 4], f32, kind="ExternalInput").ap()
    bvb = nc.dram_tensor("bvb", [128, CH], f32, kind="ExternalInput").ap()
    Sm = nc.dram_tensor("Sm", [128, 1024], f32, kind="ExternalInput").ap()
    out = nc.dram_tensor("out_partial", [T, C], f32, kind="ExternalOutput").ap()

    NT512 = T // 512          # 4   512-token stripes
    NT128 = T // 128          # 16  128-token tiles
    NC128 = C // 128          # 8   contraction tiles

    with tile.TileContext(nc) as tc:
        with tc.tile_pool(name="consts", bufs=1) as consts, \
             tc.tile_pool(name="qkv", bufs=1) as qkv, \
             tc.tile_pool(name="xp", bufs=2) as xp, \
             tc.tile_pool(name="pp", bufs=4) as pp, \
             tc.tile_pool(name="op", bufs=3) as op, \
             tc.tile_pool(name="small", bufs=2) as small, \
             tc.tile_pool(name="ps_big", bufs=3, space="PSUM") as ps_big, \
             tc.tile_pool(name="ps_s", bufs=2, space="PSUM") as ps_s, \
             tc.tile_pool(name="ps_y", bufs=2, space="PSUM") as ps_y, \
             tc.tile_pool(name="ps_b", bufs=1, space="PSUM") as ps_b:

            # ---- constants ----
            wt_sb = consts.tile([128, NC128, 3 * CH], f32r)
            nc.sync.dma_start(
                wt_sb[:], wt.rearrange("(o p) f -> p o f", p=128).bitcast(f32r))
            wpt_sb = consts.tile([128, 2, C], f32r)
            nc.sync.dma_start(
                wpt_sb[:], wpt.rearrange("(s p) o -> p s o", p=128).bitcast(f32r))
            bqk_sb = consts.tile([128, 4], f32)
            nc.sync.dma_start(bqk_sb[:], bqk)
            bvb_sb = consts.tile([128, CH], f32)
            nc.sync.dma_start(bvb_sb[:], bvb)
            S_f = consts.tile([128, 1024], f32)
            nc.sync.dma_start(S_f[:], Sm)
            S_sb = consts.tile([128, 1024], bf16)
            nc.vector.tensor_copy(S_sb[:], S_f[:])

            ones_f = consts.tile([1, 128], f32)
            nc.vector.memset(ones_f[:], 1.0)
            ones_r = consts.tile([1, 128], f32r)
            nc.vector.tensor_copy(ones_r[:], ones_f[:])
            onecol_f = consts.tile([128, 1], f32)
            nc.vector.memset(onecol_f[:], 1.0)

            # ---- persistent activations ----
            qT = qkv.tile([128, 2, T], f32r)      # [2h*64, slab, t]
            kT = qkv.tile([128, 2, T], f32r)
            vaug = qkv.tile([128, NT128, HPC, D + 1], bf16)  # [t128, ti, h, d|1]
            yT = qkv.tile([128, 2, T], f32r)

            for h in range(HPC):
                nc.vector.tensor_copy(
                    vaug[:, :, h, D:D + 1],
                    onecol_f[:].to_broadcast([128, NT128, 1]))

            for ti in range(NT512):
                # ---------- QKV projection for stripe ti ----------
                xt = xp.tile([128, NC128, 512], f32r)
                nc.sync.dma_start(
                    xt[:],
                    xT.rearrange("(o p) t -> p o t", p=128)
                      [:, :, ts(ti, 512)].bitcast(f32r))
                for fj in range(4):          # q0 q1 k0 k1
                    ps = ps_big.tile([128, 512], f32, tag="big")
                    for ci in range(NC128):
                        nc.tensor.matmul(
                            ps[:], wt_sb[:, ci, ts(fj, 128)], xt[:, ci, :],
                            start=(ci == 0), stop=(ci == NC128 - 1))
                    dest = qT if fj < 2 else kT
                    nc.vector.tensor_add(
                        out=dest[:, fj % 2, ts(ti, 512)], in0=ps[:],
                        in1=bqk_sb[:, fj:fj + 1].to_broadcast([128, 512]))
                for tj in range(4):
                    pv = ps_big.tile([128, 512], f32, tag="big")
                    for ci in range(NC128):
                        nc.tensor.matmul(
                            pv[:, :CH], xt[:, ci, ts(tj, 128)],
                            wt_sb[:, ci, 512:512 + CH],
                            start=(ci == 0), stop=(ci == NC128 - 1))
                    for h in range(HPC):
                        nc.vector.tensor_add(
                            out=vaug[:, 4 * ti + tj, h, 0:D],
                            in0=pv[:, ts(h, D)],
                            in1=bvb_sb[:, ts(h, D)])

                # ---------- attention stripe qi = ti ----------
                qi = ti
                nk = 4 * qi + 4
                for h in range(HPC):
                    hp, hs = (h % 2) * D, h // 2
                    py = ps_y.tile([D + 1, 512], f32)
                    for ki in range(nk):
                        psc = ps_s.tile([128, 512], f32)
                        nc.tensor.matmul(
                            psc[:],
                            kT[hp:hp + D, hs, ts(ki, 128)],
                            qT[hp:hp + D, hs, ts(qi, 512)],
                            start=True, stop=True)
                        p = pp.tile([128, 512], bf16)
                        nc.scalar.activation(
                            p[:], psc[:], mybir.ActivationFunctionType.Exp)
                        j = ki - 4 * qi
                        if j >= 0:  # diagonal block: causal mask
                            off = 384 - 128 * j
                            nc.vector.tensor_mul(
                                out=p[:], in0=p[:],
                                in1=S_sb[:, off:off + 512])
                        nc.tensor.matmul(
                            py[:], vaug[:, ki, h, :], p[:],
                            start=(ki == 0), stop=(ki == nk - 1))
                    # normalize: yT = py[:D] * (1/py[D]) broadcast over d
                    rec = small.tile([1, 512], f32, tag="rec")
                    nc.vector.reciprocal(rec[:], py[D:D + 1, :])
                    rec_r = small.tile([1, 512], f32r, tag="rec_r")
                    nc.vector.tensor_copy(rec_r[:], rec[:])
                    pb = ps_b.tile([D, 512], f32)
                    nc.tensor.matmul(pb[:], ones_r[:, :D], rec_r[:],
                                     start=True, stop=True)
                    bc = small.tile([D, 512], f32, tag="bc")
                    nc.vector.tensor_copy(bc[:], pb[:])
                    nc.vector.tensor_mul(
                        out=yT[hp:hp + D, hs, ts(qi, 512)],
                        in0=py[0:D, :], in1=bc[:])

                # ---------- output projection for stripe ti ----------
                for tj in range(4):
                    tg = 4 * ti + tj
                    for oi in range(2):
                        po = ps_big.tile([128, 512], f32, tag="big")
                        for s in range(2):
                            nc.tensor.matmul(
                                po[:], yT[:, s, ts(tg, 128)],
                                wpt_sb[:, s, ts(oi, 512)],
                                start=(s == 0), stop=(s == 1))
                        ot = op.tile([128, 512], f32)
                        nc.vector.tensor_copy(ot[:], po[:])
                        nc.sync.dma_start(
                            out[ts(tg, 128), ts(oi, 512)], ot[:])

    nc.compile()
    return nc


def _get_compiled():
    global _COMPILED
    if _COMPILED is None:
        _COMPILED = _build()
    return _COMPILED


def _host_prep(x, W_attn, b_attn, W_proj, b_proj):
    scale = 1.0 / np.sqrt(np.float32(D))
    xTb = [np.ascontiguousarray(x[b].T).astype(np.float32) for b in range(B)]
    Sm = (np.arange(1024, dtype=np.int32)[None, :]
          >= (np.arange(128, dtype=np.int32)[:, None] + 384)).astype(np.float32)
    in_maps = []
    for c in range(N_CORES):
        b, g = divmod(c, 4)
        ch = slice(CH * g, CH * (g + 1))
        Wq = W_attn[ch]
        Wk = W_attn[C:][ch] * scale
        Wv = W_attn[2 * C:][ch]
        wt_c = np.ascontiguousarray(
            np.concatenate([Wq, Wk, Wv], axis=0).T).astype(np.float32)
        bq = b_attn[ch]
        bk = b_attn[C:][ch] * scale
        bv = b_attn[2 * C:][ch]
        bqk_c = np.ascontiguousarray(
            np.concatenate([bq, bk]).reshape(4, 128).T).astype(np.float32)
        bvb_c = np.ascontiguousarray(
            np.broadcast_to(bv[None, :], (128, CH))).astype(np.float32)
        wpt_c = np.ascontiguousarray(W_proj[:, ch].T).astype(np.float32)
        in_maps.append({
            "xT": xTb[b],
            "wt": wt_c,
            "wpt": wpt_c,
            "bqk": bqk_c,
            "bvb": bvb_c,
            "Sm": Sm,
        })
    return in_maps


def kernel(x, W_attn, b_attn, W_proj, b_proj):
    x = np.asarray(x, dtype=np.float32)
    W_attn = np.asarray(W_attn, dtype=np.float32)
    b_attn = np.asarray(b_attn, dtype=np.float32)
    W_proj = np.asarray(W_proj, dtype=np.float32)
    b_proj = np.asarray(b_proj, dtype=np.float32)

    nc = _get_compiled()
    in_maps = _host_prep(x, W_attn, b_attn, W_proj, b_proj)
    res = run_bass_kernel_spmd(nc, in_maps, core_ids=list(range(N_CORES)))

    out = np.empty((B, T, C), dtype=np.float32)
    for b in range(B):
        acc = res.results[4 * b]["out_partial"].copy()
        for g in range(1, 4):
            acc += res.results[4 * b + g]["out_partial"]
        out[b] = acc + b_proj
    return out


# revision 9
# speedup vs baseline: 1.3927x; 1.3927x over previous
"""Causal self-attention on 8 NeuronCores (Bass/Tile, fp32r matmuls).

Sharding: tensor-parallel over heads x data-parallel over batch.
  core c -> batch b = c//4, heads 4g..4g+3 where g = c%4.
Each core computes q,k,v for its 4 heads (over its batch's 2048 tokens),
causal softmax attention in transposed-score layout [k, q] (denominator via
an extra ones-column on v), and the partial output projection over its 256
head-channels. Host sums the 4 partials per batch and adds b_proj.

Matmuls run as float32r (full PE rate at N>=256, ~1e-4 relative rounding);
the attention probabilities p and values v are bf16 (DVE 2x/4x modes; the
softmax numerator and denominator use the same rounded p, so the error
largely cancels). The 1/sqrt(d) scale is folded into W_k/b_k on the host.

The per-512-token stripes are emitted interleaved (qkv stripe ti, then
attention stripe qi=ti, then its output projection) so the Tile scheduler
overlaps PE-heavy projection work with ACT-heavy softmax work.
"""

import os
import sys

for _p in ("/opt/trn_rl_repo", "/opt/pypackages"):
    if os.path.isdir(_p) and _p not in sys.path:
        sys.path.append(_p)

import numpy as np

import concourse.bass as bass
import concourse.tile as tile
import concourse.mybir as mybir
from concourse import bacc
from concourse.bass_utils import run_bass_kernel_spmd

B, T, C = 2, 2048, 1024
H = 16            # total heads
D = 64            # head dim
HPC = 4           # heads per core
CH = HPC * D      # 256 channels per core
N_CORES = 8

f32 = mybir.dt.float32
f32r = mybir.dt.float32r
bf16 = mybir.dt.bfloat16
ts = bass.ts

_COMPILED = None


def _build():
    nc = bacc.Bacc("TRN2", target_bir_lowering=False, debug=False,
                   num_devices=N_CORES)

    xT = nc.dram_tensor("xT", [C, T], f32, kind="ExternalInput").ap()
    wt = nc.dram_tensor("wt", [C, 3 * CH], f32, kind="ExternalInput").ap()
    wpt = nc.dram_tensor("wpt", [CH, C], f32, kind="ExternalInput").ap()
    bqk = nc.dram_tensor("bqk", [128, 4], f32, kind="ExternalInput").ap()
    bvb = nc.dram_tensor("bvb", [128, CH], f32, kind="ExternalInput").ap()
    Sm = nc.dram_tensor("Sm", [128, 1024], f32, kind="ExternalInput").ap()
    out = nc.dram_tensor("out_partial", [T, C], f32, kind="ExternalOutput").ap()

    NT512 = T // 512          # 4   512-token stripes
    NT128 = T // 128          # 16  128-token tiles
    NC128 = C // 128          # 8   contraction tiles

    with tile.TileContext(nc) as tc:
        with tc.tile_pool(name="consts", bufs=1) as consts, \
             tc.tile_pool(name="qkv", bufs=1) as qkv, \
             tc.tile_pool(name="xp", bufs=2) as xp, \
             tc.tile_pool(name="pp", bufs=4) as pp, \
             tc.tile_pool(name="op", bufs=3) as op, \
             tc.tile_pool(name="small", bufs=2) as small, \
             tc.tile_pool(name="ps_big", bufs=2, space="PSUM") as ps_big, \
             tc.tile_pool(name="ps_s", bufs=2, space="PSUM") as ps_s, \
             tc.tile_pool(name="ps_y", bufs=2, space="PSUM") as ps_y, \
             tc.tile_pool(name="ps_o", bufs=2, space="PSUM") as ps_o:

            # ---- constants (split DMAs so the first matmuls start early) ----
            wt_r = wt.rearrange("(o p) f -> p o f", p=128).bitcast(f32r)
            wt_sb = consts.tile([128, NC128, 3 * CH], f32r)
            for ci in range(NC128):
                nc.sync.dma_start(wt_sb[:, ci], wt_r[:, ci])
            wpt_sb = consts.tile([128, 2, C], f32r)
            nc.sync.dma_start(
                wpt_sb[:], wpt.rearrange("(s p) o -> p s o", p=128).bitcast(f32r))
            bqk_sb = consts.tile([128, 4], f32)
            nc.sync.dma_start(bqk_sb[:], bqk)
            bvb_sb = consts.tile([128, CH], f32)
            nc.sync.dma_start(bvb_sb[:], bvb)
            S_f = consts.tile([128, 1024], f32)
            nc.sync.dma_start(S_f[:], Sm)
            S_sb = consts.tile([128, 1024], bf16)
            nc.vector.tensor_copy(S_sb[:], S_f[:])

            onecol_f = consts.tile([128, 1], f32)
            nc.vector.memset(onecol_f[:], 1.0)

            # ---- persistent activations ----
            qT = qkv.tile([128, 2, T], f32r)      # [2h*64, slab, t]
            kT = qkv.tile([128, 2, T], f32r)
            vaug = qkv.tile([128, NT128, HPC, D + 1], bf16)  # [t128, ti, h, d|1]
            yT = qkv.tile([128, 2, T], f32r)

            for h in range(HPC):
                nc.vector.tensor_copy(
                    vaug[:, :, h, D:D + 1],
                    onecol_f[:].to_broadcast([128, NT128, 1]))

            xT_r = xT.rearrange("(o p) t -> p o t", p=128).bitcast(f32r)
            for ti in range(NT512):
                # ---------- QKV projection for stripe ti ----------
                xt = xp.tile([128, NC128, 512], f32r)
                for ci in range(NC128):
                    nc.sync.dma_start(xt[:, ci], xT_r[:, ci, ts(ti, 512)])
                for fj in range(4):          # q0 q1 k0 k1
                    ps = ps_big.tile([128, 512], f32, tag="big")
                    for ci in range(NC128):
                        nc.tensor.matmul(
                            ps[:], wt_sb[:, ci, ts(fj, 128)], xt[:, ci, :],
                            start=(ci == 0), stop=(ci == NC128 - 1))
                    dest = qT if fj < 2 else kT
                    nc.vector.tensor_add(
                        out=dest[:, fj % 2, ts(ti, 512)], in0=ps[:],
                        in1=bqk_sb[:, fj:fj + 1].to_broadcast([128, 512]))
                for tj in range(4):
                    pv = ps_big.tile([128, 512], f32, tag="big")
                    for ci in range(NC128):
                        nc.tensor.matmul(
                            pv[:, :CH], xt[:, ci, ts(tj, 128)],
                            wt_sb[:, ci, 512:512 + CH],
                            start=(ci == 0), stop=(ci == NC128 - 1))
                    for h in range(HPC):
                        nc.vector.tensor_add(
                            out=vaug[:, 4 * ti + tj, h, 0:D],
                            in0=pv[:, ts(h, D)],
                            in1=bvb_sb[:, ts(h, D)])

                # ---------- attention stripe qi = ti ----------
                qi = ti
                nk = 4 * qi + 4
                for h in range(HPC):
                    hp, hs = (h % 2) * D, h // 2
                    py = ps_y.tile([D + 1, 512], f32)
                    for ki in range(nk):
                        j = ki - 4 * qi
                        # columns qq < 128*j of this stripe are fully masked
                        q0 = max(0, 128 * j)
                        w = 512 - q0
                        psc = ps_s.tile([128, 512], f32)
                        nc.tensor.matmul(
                            psc[:, q0:],
                            kT[hp:hp + D, hs, ts(ki, 128)],
                            qT[hp:hp + D, hs, bass.ds(512 * qi + q0, w)],
                            start=True, stop=True)
                        p = pp.tile([128, 512], bf16)
                        nc.scalar.activation(
                            p[:, q0:], psc[:, q0:],
                            mybir.ActivationFunctionType.Exp)
                        if j >= 0:  # partial 128 columns need the causal mask
                            nc.vector.tensor_mul(
                                out=p[:, q0:q0 + 128], in0=p[:, q0:q0 + 128],
                                in1=S_sb[:, 384:512])
                        nc.tensor.matmul(
                            py[:, q0:], vaug[:, ki, h, :], p[:, q0:],
                            start=(ki == 0), stop=(ki == nk - 1))
                    # normalize: yT = py[:D] * (1/py[D]) broadcast over d
                    rec = small.tile([1, 512], f32, tag="rec")
                    nc.vector.reciprocal(rec[:], py[D:D + 1, :])
                    bc = small.tile([D, 512], f32, tag="bc")
                    nc.gpsimd.partition_broadcast(bc[:], rec[:], channels=D)
                    nc.vector.tensor_mul(
                        out=yT[hp:hp + D, hs, ts(qi, 512)],
                        in0=py[0:D, :], in1=bc[:])

                # ---------- output projection for stripe ti ----------
                for tj in range(4):
                    tg = 4 * ti + tj
                    for oi in range(2):
                        po = ps_o.tile([128, 512], f32, tag="po")
                        for s in range(2):
                            nc.tensor.matmul(
                                po[:], yT[:, s, ts(tg, 128)],
                                wpt_sb[:, s, ts(oi, 512)],
                                start=(s == 0), stop=(s == 1))
                        ot = op.tile([128, 512], f32)
                        nc.vector.tensor_copy(ot[:], po[:])
                        nc.sync.dma_start(
                            out[ts(tg, 128), ts(oi, 512)], ot[:])

    nc.compile()
    return nc


def _get_compiled():
    global _COMPILED
    if _COMPILED is None:
        _COMPILED = _build()
    return _COMPILED


def _host_prep(x, W_attn, b_attn, W_proj, b_proj):
    scale = 1.0 / np.sqrt(np.float32(D))
    xTb = [np.ascontiguousarray(x[b].T).astype(np.float32) for b in range(B)]
    Sm = (np.arange(1024, dtype=np.int32)[None, :]
          >= (np.arange(128, dtype=np.int32)[:, None] + 384)).astype(np.float32)
    in_maps = []
    for c in range(N_CORES):
        b, g = divmod(c, 4)
        ch = slice(CH * g, CH * (g + 1))
        Wq = W_attn[ch]
        Wk = W_attn[C:][ch] * scale
        Wv = W_attn[2 * C:][ch]
        wt_c = np.ascontiguousarray(
            np.concatenate([Wq, Wk, Wv], axis=0).T).astype(np.float32)
        bq = b_attn[ch]
        bk = b_attn[C:][ch] * scale
        bv = b_attn[2 * C:][ch]
        bqk_c = np.ascontiguousarray(
            np.concatenate([bq, bk]).reshape(4, 128).T).astype(np.float32)
        bvb_c = np.ascontiguousarray(
            np.broadcast_to(bv[None, :], (128, CH))).astype(np.float32)
        wpt_c = np.ascontiguousarray(W_proj[:, ch].T).astype(np.float32)
        in_maps.append({
            "xT": xTb[b],
            "wt": wt_c,
            "wpt": wpt_c,
            "bqk": bqk_c,
            "bvb": bvb_c,
            "Sm": Sm,
        })
    return in_maps


def kernel(x, W_attn, b_attn, W_proj, b_proj):
    x = np.asarray(x, dtype=np.float32)
    W_attn = np.asarray(W_attn, dtype=np.float32)
    b_attn = np.asarray(b_attn, dtype=np.float32)
    W_proj = np.asarray(W_proj, dtype=np.float32)
    b_proj = np.asarray(b_proj, dtype=np.float32)

    nc = _get_compiled()
    in_maps = _host_prep(x, W_attn, b_attn, W_proj, b_proj)
    res = run_bass_kernel_spmd(nc, in_maps, core_ids=list(range(N_CORES)))

    out = np.empty((B, T, C), dtype=np.float32)
    for b in range(B):
        acc = res.results[4 * b]["out_partial"].copy()
        for g in range(1, 4):
            acc += res.results[4 * b + g]["out_partial"]
        out[b] = acc + b_proj
    return out


# revision 30
# speedup vs baseline: 1.6082x; 1.1547x over previous
"""Causal self-attention on 8 NeuronCores (Bass/Tile, fp32r matmuls).

Sharding: tensor-parallel over heads x data-parallel over batch.
  core c -> batch b = c//4, heads 4g..4g+3 where g = c%4.
Each core computes q,k,v for its 4 heads (over its batch's 2048 tokens),
causal softmax attention in transposed-score layout [k, q] (denominator via
an extra ones-column on v), and the partial output projection over its 256
head-channels. Host sums the 4 partials per batch and adds b_proj.

Matmuls run as float32r (full PE rate at N>=256, ~1e-4 relative rounding);
the attention probabilities p and values v are bf16 (DVE 2x/4x modes; the
softmax numerator and denominator use the same rounded p, so the error
largely cancels). The 1/sqrt(d) scale is folded into W_k/b_k on the host.

The per-512-token stripes are emitted interleaved (qkv stripe ti, then
attention stripe qi=ti) so the Tile scheduler overlaps PE-heavy projection
work with ACT-heavy softmax work; the output projection is emitted last so
its PE work fills the ACT-bound tail of the late (long) attention stripes.
Diagonal score blocks are narrowed to skip fully-masked columns.
"""

import os
import sys

for _p in ("/opt/trn_rl_repo", "/opt/pypackages"):
    if os.path.isdir(_p) and _p not in sys.path:
        sys.path.append(_p)

import numpy as np

import concourse.bass as bass
import concourse.tile as tile
import concourse.mybir as mybir
from concourse import bacc
from concourse.bass_utils import run_bass_kernel_spmd

B, T, C = 2, 2048, 1024
H = 16            # total heads
D = 64            # head dim
HPC = 4           # heads per core
CH = HPC * D      # 256 channels per core
N_CORES = 8

f32 = mybir.dt.float32
f32r = mybir.dt.float32r
bf16 = mybir.dt.bfloat16
ts = bass.ts

_COMPILED = None


def _build():
    nc = bacc.Bacc("TRN2", target_bir_lowering=False, debug=False,
                   num_devices=N_CORES)

    xT = nc.dram_tensor("xT", [C, T], f32, kind="ExternalInput").ap()
    wt = nc.dram_tensor("wt", [C, 3 * CH], f32, kind="ExternalInput").ap()
    wpt = nc.dram_tensor("wpt", [CH, C], f32, kind="ExternalInput").ap()
    bqk = nc.dram_tensor("bqk", [128, 4], f32, kind="ExternalInput").ap()
    bvb = nc.dram_tensor("bvb", [128, CH], f32, kind="ExternalInput").ap()
    Sm = nc.dram_tensor("Sm", [128, 1024], f32, kind="ExternalInput").ap()
    out = nc.dram_tensor("out_partial", [T, C], f32, kind="ExternalOutput").ap()

    NT512 = T // 512          # 4   512-token stripes
    NT128 = T // 128          # 16  128-token tiles
    NC128 = C // 128          # 8   contraction tiles

    with tile.TileContext(nc) as tc:
        with tc.tile_pool(name="consts", bufs=1) as consts, \
             tc.tile_pool(name="qkv", bufs=1) as qkv, \
             tc.tile_pool(name="xp", bufs=3) as xp, \
             tc.tile_pool(name="pp", bufs=6) as pp, \
             tc.tile_pool(name="op", bufs=3) as op, \
             tc.tile_pool(name="small", bufs=2) as small, \
             tc.tile_pool(name="ps_big", bufs=2, space="PSUM") as ps_big, \
             tc.tile_pool(name="ps_s", bufs=3, space="PSUM") as ps_s, \
             tc.tile_pool(name="ps_y", bufs=1, space="PSUM") as ps_y, \
             tc.tile_pool(name="ps_o", bufs=2, space="PSUM") as ps_o:

            # ---- constants; DMA emission order is chosen so the first
            #      qk matmul chains of stripe 0 can start as early as
            #      possible: interleave xt(0)[ci] with the qk half of
            #      wt[ci], defer the v-half / masks / wpt ----
            xT_r = xT.rearrange("(o p) t -> p o t", p=128).bitcast(f32r)
            wt_r = wt.rearrange("(o p) f -> p o f", p=128).bitcast(f32r)
            wt_sb = consts.tile([128, NC128, 3 * CH], f32r)
            xt0 = xp.tile([128, NC128, 512], f32r, tag="xt")
            for ci in range(NC128):
                nc.sync.dma_start(xt0[:, ci], xT_r[:, ci, ts(0, 512)])
                nc.sync.dma_start(wt_sb[:, ci, :512], wt_r[:, ci, :512])
            bqk_sb = consts.tile([128, 4], f32)
            nc.sync.dma_start(bqk_sb[:], bqk)
            for ci in range(NC128):
                nc.sync.dma_start(wt_sb[:, ci, 512:], wt_r[:, ci, 512:])
            bvb_sb = consts.tile([128, CH], f32)
            nc.sync.dma_start(bvb_sb[:], bvb)
            S_f = consts.tile([128, 1024], f32)
            nc.sync.dma_start(S_f[:], Sm)
            S_sb = consts.tile([128, 1024], bf16)
            nc.vector.tensor_copy(S_sb[:], S_f[:])
            wpt_sb = consts.tile([128, 2, C], f32r)
            nc.sync.dma_start(
                wpt_sb[:], wpt.rearrange("(s p) o -> p s o", p=128).bitcast(f32r))

            onecol_f = consts.tile([128, 1], f32)
            nc.vector.memset(onecol_f[:], 1.0)

            # ---- persistent activations ----
            qT = qkv.tile([128, 2, T], f32r)      # [2h*64, slab, t]
            kT = qkv.tile([128, 2, T], f32r)
            vaug = qkv.tile([128, NT128, HPC, D + 1], bf16)  # [t128, ti, h, d|1]
            yT = qkv.tile([128, 2, T], f32r)

            for h in range(HPC):
                nc.vector.tensor_copy(
                    vaug[:, :, h, D:D + 1],
                    onecol_f[:].to_broadcast([128, NT128, 1]))

            for ti in range(NT512):
                # ---------- QKV projection for stripe ti ----------
                if ti == 0:
                    xt = xt0
                else:
                    xt = xp.tile([128, NC128, 512], f32r, tag="xt")
                    for ci in range(NC128):
                        nc.sync.dma_start(xt[:, ci], xT_r[:, ci, ts(ti, 512)])
                for fj in range(4):          # q0 q1 k0 k1
                    ps = ps_big.tile([128, 512], f32, tag="big")
                    for ci in range(NC128):
                        nc.tensor.matmul(
                            ps[:], wt_sb[:, ci, ts(fj, 128)], xt[:, ci, :],
                            start=(ci == 0), stop=(ci == NC128 - 1))
                    dest = qT if fj < 2 else kT
                    nc.vector.tensor_add(
                        out=dest[:, fj % 2, ts(ti, 512)], in0=ps[:],
                        in1=bqk_sb[:, fj:fj + 1].to_broadcast([128, 512]))
                for tj in range(4):
                    pv = ps_big.tile([128, 512], f32, tag="big")
                    for ci in range(NC128):
                        nc.tensor.matmul(
                            pv[:, :CH], xt[:, ci, ts(tj, 128)],
                            wt_sb[:, ci, 512:512 + CH],
                            start=(ci == 0), stop=(ci == NC128 - 1))
                    for h in range(HPC):
                        nc.vector.tensor_add(
                            out=vaug[:, 4 * ti + tj, h, 0:D],
                            in0=pv[:, ts(h, D)],
                            in1=bvb_sb[:, ts(h, D)])

                # ---------- attention stripe qi = ti ----------
                qi = ti
                nk = 4 * qi + 4
                for h in range(HPC):
                    hp, hs = (h % 2) * D, h // 2
                    py = ps_y.tile([D + 1, 512], f32)
                    for ki in range(nk):
                        j = ki - 4 * qi
                        # columns qq < 128*j of this stripe are fully masked
                        q0 = max(0, 128 * j)
                        w = 512 - q0
                        psc = ps_s.tile([128, 512], f32)
                        nc.tensor.matmul(
                            psc[:, q0:],
                            kT[hp:hp + D, hs, ts(ki, 128)],
                            qT[hp:hp + D, hs, bass.ds(512 * qi + q0, w)],
                            start=True, stop=True)
                        p = pp.tile([128, 512], bf16)
                        nc.scalar.activation(
                            p[:, q0:], psc[:, q0:],
                            mybir.ActivationFunctionType.Exp)
                        if j >= 0:  # partial 128 columns need the causal mask
                            nc.vector.tensor_mul(
                                out=p[:, q0:q0 + 128], in0=p[:, q0:q0 + 128],
                                in1=S_sb[:, 384:512])
                        nc.tensor.matmul(
                            py[:, q0:], vaug[:, ki, h, :], p[:, q0:],
                            start=(ki == 0), stop=(ki == nk - 1))
                    # normalize: yT = py[:D] * (1/py[D]) broadcast over d
                    rec = small.tile([1, 512], f32, tag="rec")
                    nc.vector.reciprocal(rec[:], py[D:D + 1, :])
                    bc = small.tile([D, 512], f32, tag="bc")
                    nc.gpsimd.partition_broadcast(bc[:], rec[:], channels=D)
                    nc.vector.tensor_mul(
                        out=yT[hp:hp + D, hs, ts(qi, 512)],
                        in0=py[0:D, :], in1=bc[:])

            # ---------- output projection (emitted last so its PE work
            #            fills the ACT-bound tail of late attention stripes) --
            for tg in range(NT128):
                for oi in range(2):
                    po = ps_o.tile([128, 512], f32, tag="po")
                    for s in range(2):
                        nc.tensor.matmul(
                            po[:], yT[:, s, ts(tg, 128)],
                            wpt_sb[:, s, ts(oi, 512)],
                            start=(s == 0), stop=(s == 1))
                    ot = op.tile([128, 512], f32)
                    nc.vector.tensor_copy(ot[:], po[:])
                    nc.sync.dma_start(
                        out[ts(tg, 128), ts(oi, 512)], ot[:])

    nc.compile()
    return nc


def _get_compiled():
    global _COMPILED
    if _COMPILED is None:
        _COMPILED = _build()
    return _COMPILED


def _host_prep(x, W_attn, b_attn, W_proj, b_proj):
    scale = 1.0 / np.sqrt(np.float32(D))
    xTb = [np.ascontiguousarray(x[b].T).astype(np.float32) for b in range(B)]
    Sm = (np.arange(1024, dtype=np.int32)[None, :]
          >= (np.arange(128, dtype=np.int32)[:, None] + 384)).astype(np.float32)
    in_maps = []
    for c in range(N_CORES):
        b, g = divmod(c, 4)
        ch = slice(CH * g, CH * (g + 1))
        Wq = W_attn[ch]
        Wk = W_attn[C:][ch] * scale
        Wv = W_attn[2 * C:][ch]
        wt_c = np.ascontiguousarray(
            np.concatenate([Wq, Wk, Wv], axis=0).T).astype(np.float32)
        bq = b_attn[ch]
        bk = b_attn[C:][ch] * scale
        bv = b_attn[2 * C:][ch]
        bqk_c = np.ascontiguousarray(
            np.concatenate([bq, bk]).reshape(4, 128).T).astype(np.float32)
        bvb_c = np.ascontiguousarray(
            np.broadcast_to(bv[None, :], (128, CH))).astype(np.float32)
        wpt_c = np.ascontiguousarray(W_proj[:, ch].T).astype(np.float32)
        in_maps.append({
            "xT": xTb[b],
            "wt": wt_c,
            "wpt": wpt_c,
            "bqk": bqk_c,
            "bvb": bvb_c,
            "Sm": Sm,
        })
    return in_maps


def kernel(x, W_attn, b_attn, W_proj, b_proj):
    x = np.asarray(x, dtype=np.float32)
    W_attn = np.asarray(W_attn, dtype=np.float32)
    b_attn = np.asarray(b_attn, dtype=np.float32)
    W_proj = np.asarray(W_proj, dtype=np.float32)
    b_proj = np.asarray(b_proj, dtype=np.float32)

    nc = _get_compiled()
    in_maps = _host_prep(x, W_attn, b_attn, W_proj, b_proj)
    res = run_bass_kernel_spmd(nc, in_maps, core_ids=list(range(N_CORES)))

    out = np.empty((B, T, C), dtype=np.float32)
    for b in range(B):
        acc = res.results[4 * b]["out_partial"].copy()
        for g in range(1, 4):
            acc += res.results[4 * b + g]["out_partial"]
        out[b] = acc + b_proj
    return out


# revision 31
# speedup vs baseline: 1.6401x; 1.0199x over previous
"""Causal self-attention on 8 NeuronCores (Bass/Tile, fp32r matmuls).

Sharding: tensor-parallel over heads x data-parallel over batch.
  core c -> batch b = c//4, heads 4g..4g+3 where g = c%4.
Each core computes q,k,v for its 4 heads (over its batch's 2048 tokens),
causal softmax attention in transposed-score layout [k, q] (denominator via
an extra ones-column on v), and the partial output projection over its 256
head-channels. Host sums the 4 partials per batch and adds b_proj.

Matmuls run as float32r (full PE rate at N>=256, ~1e-4 relative rounding);
the attention probabilities p and values v are bf16 (DVE 2x/4x modes; the
softmax numerator and denominator use the same rounded p, so the error
largely cancels). The 1/sqrt(d) scale is folded into W_k/b_k on the host.

The per-512-token stripes are emitted interleaved (qkv stripe ti, then
attention stripe qi=ti) so the Tile scheduler overlaps PE-heavy projection
work with ACT-heavy softmax work; the output projection is emitted last so
its PE work fills the ACT-bound tail of the late (long) attention stripes.
Diagonal score blocks are narrowed to skip fully-masked columns.
"""

import os
import sys

for _p in ("/opt/trn_rl_repo", "/opt/pypackages"):
    if os.path.isdir(_p) and _p not in sys.path:
        sys.path.append(_p)

import numpy as np

import concourse.bass as bass
import concourse.tile as tile
import concourse.mybir as mybir
from concourse import bacc
from concourse.bass_utils import run_bass_kernel_spmd

B, T, C = 2, 2048, 1024
H = 16            # total heads
D = 64            # head dim
HPC = 4           # heads per core
CH = HPC * D      # 256 channels per core
N_CORES = 8

f32 = mybir.dt.float32
f32r = mybir.dt.float32r
bf16 = mybir.dt.bfloat16
ts = bass.ts

_COMPILED = None


def _build():
    nc = bacc.Bacc("TRN2", target_bir_lowering=False, debug=False,
                   num_devices=N_CORES)

    xT = nc.dram_tensor("xT", [C, T], f32, kind="ExternalInput").ap()
    wt = nc.dram_tensor("wt", [C, 3 * CH], f32, kind="ExternalInput").ap()
    wpt = nc.dram_tensor("wpt", [CH, C], f32, kind="ExternalInput").ap()
    bqk = nc.dram_tensor("bqk", [128, 4], f32, kind="ExternalInput").ap()
    bvb = nc.dram_tensor("bvb", [128, CH], f32, kind="ExternalInput").ap()
    Sm = nc.dram_tensor("Sm", [128, 1024], f32, kind="ExternalInput").ap()
    out = nc.dram_tensor("out_partial", [T, C], f32, kind="ExternalOutput").ap()

    NT512 = T // 512          # 4   512-token stripes
    NT128 = T // 128          # 16  128-token tiles
    NC128 = C // 128          # 8   contraction tiles

    with tile.TileContext(nc) as tc:
        with tc.tile_pool(name="consts", bufs=1) as consts, \
             tc.tile_pool(name="qkv", bufs=1) as qkv, \
             tc.tile_pool(name="xp", bufs=3) as xp, \
             tc.tile_pool(name="pp", bufs=8) as pp, \
             tc.tile_pool(name="op", bufs=4) as op, \
             tc.tile_pool(name="small", bufs=2) as small, \
             tc.tile_pool(name="ps_big", bufs=2, space="PSUM") as ps_big, \
             tc.tile_pool(name="ps_s", bufs=3, space="PSUM") as ps_s, \
             tc.tile_pool(name="ps_y", bufs=1, space="PSUM") as ps_y, \
             tc.tile_pool(name="ps_o", bufs=2, space="PSUM") as ps_o:

            # ---- constants; DMA emission order is chosen so the first
            #      qk matmul chains of stripe 0 can start as early as
            #      possible: interleave xt(0)[ci] with the qk half of
            #      wt[ci], defer the v-half / masks / wpt ----
            xT_r = xT.rearrange("(o p) t -> p o t", p=128).bitcast(f32r)
            wt_r = wt.rearrange("(o p) f -> p o f", p=128).bitcast(f32r)
            wt_sb = consts.tile([128, NC128, 3 * CH], f32r)
            xt0 = xp.tile([128, NC128, 512], f32r, tag="xt")
            for ci in range(NC128):
                nc.sync.dma_start(xt0[:, ci], xT_r[:, ci, ts(0, 512)])
                nc.sync.dma_start(wt_sb[:, ci, :512], wt_r[:, ci, :512])
            bqk_sb = consts.tile([128, 4], f32)
            nc.sync.dma_start(bqk_sb[:], bqk)
            for ci in range(NC128):
                nc.sync.dma_start(wt_sb[:, ci, 512:], wt_r[:, ci, 512:])
            bvb_sb = consts.tile([128, CH], f32)
            nc.sync.dma_start(bvb_sb[:], bvb)
            S_f = consts.tile([128, 1024], f32)
            nc.sync.dma_start(S_f[:], Sm)
            S_sb = consts.tile([128, 1024], bf16)
            nc.vector.tensor_copy(S_sb[:], S_f[:])
            wpt_sb = consts.tile([128, 2, C], f32r)
            nc.sync.dma_start(
                wpt_sb[:], wpt.rearrange("(s p) o -> p s o", p=128).bitcast(f32r))

            onecol_f = consts.tile([128, 1], f32)
            nc.vector.memset(onecol_f[:], 1.0)

            # ---- persistent activations ----
            qT = qkv.tile([128, 2, T], f32r)      # [2h*64, slab, t]
            kT = qkv.tile([128, 2, T], f32r)
            vaug = qkv.tile([128, NT128, HPC, D + 1], bf16)  # [t128, ti, h, d|1]
            yT = qkv.tile([128, 2, T], f32r)

            for h in range(HPC):
                nc.vector.tensor_copy(
                    vaug[:, :, h, D:D + 1],
                    onecol_f[:].to_broadcast([128, NT128, 1]))

            for ti in range(NT512):
                # ---------- QKV projection for stripe ti ----------
                if ti == 0:
                    xt = xt0
                else:
                    xt = xp.tile([128, NC128, 512], f32r, tag="xt")
                    for ci in range(NC128):
                        nc.sync.dma_start(xt[:, ci], xT_r[:, ci, ts(ti, 512)])
                for fj in range(4):          # q0 q1 k0 k1
                    ps = ps_big.tile([128, 512], f32, tag="big")
                    for ci in range(NC128):
                        nc.tensor.matmul(
                            ps[:], wt_sb[:, ci, ts(fj, 128)], xt[:, ci, :],
                            start=(ci == 0), stop=(ci == NC128 - 1))
                    dest = qT if fj < 2 else kT
                    nc.vector.tensor_add(
                        out=dest[:, fj % 2, ts(ti, 512)], in0=ps[:],
                        in1=bqk_sb[:, fj:fj + 1].to_broadcast([128, 512]))
                for tj in range(4):
                    pv = ps_big.tile([128, 512], f32, tag="big")
                    for ci in range(NC128):
                        nc.tensor.matmul(
                            pv[:, :CH], xt[:, ci, ts(tj, 128)],
                            wt_sb[:, ci, 512:512 + CH],
                            start=(ci == 0), stop=(ci == NC128 - 1))
                    for h in range(HPC):
                        nc.vector.tensor_add(
                            out=vaug[:, 4 * ti + tj, h, 0:D],
                            in0=pv[:, ts(h, D)],
                            in1=bvb_sb[:, ts(h, D)])

                # ---------- attention stripe qi = ti ----------
                qi = ti
                nk = 4 * qi + 4
                for h in range(HPC):
                    hp, hs = (h % 2) * D, h // 2
                    py = ps_y.tile([D + 1, 512], f32)
                    for ki in range(nk):
                        j = ki - 4 * qi
                        # columns qq < 128*j of this stripe are fully masked
                        q0 = max(0, 128 * j)
                        w = 512 - q0
                        psc = ps_s.tile([128, 512], f32)
                        nc.tensor.matmul(
                            psc[:, q0:],
                            kT[hp:hp + D, hs, ts(ki, 128)],
                            qT[hp:hp + D, hs, bass.ds(512 * qi + q0, w)],
                            start=True, stop=True)
                        p = pp.tile([128, 512], bf16)
                        nc.scalar.activation(
                            p[:, q0:], psc[:, q0:],
                            mybir.ActivationFunctionType.Exp)
                        if j >= 0:  # partial 128 columns need the causal mask
                            nc.vector.tensor_mul(
                                out=p[:, q0:q0 + 128], in0=p[:, q0:q0 + 128],
                                in1=S_sb[:, 384:512])
                        nc.tensor.matmul(
                            py[:, q0:], vaug[:, ki, h, :], p[:, q0:],
                            start=(ki == 0), stop=(ki == nk - 1))
                    # normalize: yT = py[:D] * (1/py[D]) broadcast over d
                    rec = small.tile([1, 512], f32, tag="rec")
                    nc.vector.reciprocal(rec[:], py[D:D + 1, :])
                    bc = small.tile([D, 512], f32, tag="bc")
                    nc.gpsimd.partition_broadcast(bc[:], rec[:], channels=D)
                    nc.vector.tensor_mul(
                        out=yT[hp:hp + D, hs, ts(qi, 512)],
                        in0=py[0:D, :], in1=bc[:])

            # ---------- output projection (emitted last so its PE work
            #            fills the ACT-bound tail of late attention stripes) --
            for tg in range(NT128):
                for oi in range(2):
                    po = ps_o.tile([128, 512], f32, tag="po")
                    for s in range(2):
                        nc.tensor.matmul(
                            po[:], yT[:, s, ts(tg, 128)],
                            wpt_sb[:, s, ts(oi, 512)],
                            start=(s == 0), stop=(s == 1))
                    ot = op.tile([128, 512], f32)
                    nc.vector.tensor_copy(ot[:], po[:])
                    nc.sync.dma_start(
                        out[ts(tg, 128), ts(oi, 512)], ot[:])

    nc.compile()
    return nc


def _get_compiled():
    global _COMPILED
    if _COMPILED is None:
        _COMPILED = _build()
    return _COMPILED


def _host_prep(x, W_attn, b_attn, W_proj, b_proj):
    scale = 1.0 / np.sqrt(np.float32(D))
    xTb = [np.ascontiguousarray(x[b].T).astype(np.float32) for b in range(B)]
    Sm = (np.arange(1024, dtype=np.int32)[None, :]
          >= (np.arange(128, dtype=np.int32)[:, None] + 384)).astype(np.float32)
    in_maps = []
    for c in range(N_CORES):
        b, g = divmod(c, 4)
        ch = slice(CH * g, CH * (g + 1))
        Wq = W_attn[ch]
        Wk = W_attn[C:][ch] * scale
        Wv = W_attn[2 * C:][ch]
        wt_c = np.ascontiguousarray(
            np.concatenate([Wq, Wk, Wv], axis=0).T).astype(np.float32)
        bq = b_attn[ch]
        bk = b_attn[C:][ch] * scale
        bv = b_attn[2 * C:][ch]
        bqk_c = np.ascontiguousarray(
            np.concatenate([bq, bk]).reshape(4, 128).T).astype(np.float32)
        bvb_c = np.ascontiguousarray(
            np.broadcast_to(bv[None, :], (128, CH))).astype(np.float32)
        wpt_c = np.ascontiguousarray(W_proj[:, ch].T).astype(np.float32)
        in_maps.append({
            "xT": xTb[b],
            "wt": wt_c,
            "wpt": wpt_c,
            "bqk": bqk_c,
            "bvb": bvb_c,
            "Sm": Sm,
        })
    return in_maps


def kernel(x, W_attn, b_attn, W_proj, b_proj):
    x = np.asarray(x, dtype=np.float32)
    W_attn = np.asarray(W_attn, dtype=np.float32)
    b_attn = np.asarray(b_attn, dtype=np.float32)
    W_proj = np.asarray(W_proj, dtype=np.float32)
    b_proj = np.asarray(b_proj, dtype=np.float32)

    nc = _get_compiled()
    in_maps = _host_prep(x, W_attn, b_attn, W_proj, b_proj)
    res = run_bass_kernel_spmd(nc, in_maps, core_ids=list(range(N_CORES)))

    out = np.empty((B, T, C), dtype=np.float32)
    for b in range(B):
        acc = res.results[4 * b]["out_partial"].copy()
        for g in range(1, 4):
            acc += res.results[4 * b + g]["out_partial"]
        out[b] = acc + b_proj
    return out
